# revision 1
# baseline (speedup 1.0000x reference)
"""Trainium2 Bass kernel for ViTDet-style attention with decomposed relative
position bias (B=8, H=W=32, dim=768, 12 heads).

Strategy
--------
Data-parallel over the batch: each of the 8 NeuronCores processes one batch
element end-to-end (qkv projection, biased attention, output projection).

The decomposed rel-pos bias is folded into the QK^T matmul by augmenting the
per-head contraction dimension from 64 to exactly 128:
    K_aug = [ onehot_h (32) ; onehot_w (32) ; k^T (64) ]
    Q_aug = [ (q @ Rh)^T (32) ; (q @ Rw)^T (32) ; q^T (64) ]
so S^T = scale*(q.k) + rel_h + rel_w in ONE K=128 matmul per tile.

All matmul operands are bf16 (fast weight loads, half the DMA bytes); PSUM
accumulation stays fp32.  The logit scale folds 1/(8*16) into W_q so the
matmul produces S/16 directly:
 - the Scalar engine computes exp via ACTIVATE(Exp, scale=16)
 - the Vector engine computes exp via a custom 8-stage DVE op:
       exp16(z) = (1 + z*(c0 + z*c1))^16        (z = S/16, |z| <= ~0.19)
   a degree-2 seed + 4 squarings, max rel err ~0.5% over |S|<=3.
Splitting exp across both engines removes the Scalar-engine bottleneck that
limits an ACT-only softmax to ~128us.

Attention runs transposed (keys on partitions) so exp output feeds A@V with
no transposes; a ones-column on V gives softmax row-sums for free.  The
normalization reciprocal is broadcast across partitions with a tiny
ones-vector matmul on the idle PE (PSUM rows 64:128 of the same A@V tile)
instead of the slow GpSimd partition_broadcast.

Weight layouts are head-major ([q_h | k_h] per 128-wide tile) so every
PSUM->SBUF copy lands partition-aligned except q (Scalar engine handles the
cross-partition halves, proven on this hardware).

Bias handling (all exact):
 - k-bias adds a per-query constant to all key logits -> cancels in softmax.
 - v-bias and proj-bias contribute `qkv_b[v] @ proj_w + proj_b` to every
   output row (softmax rows sum to 1); added on the host after gather.
 - q-bias would need an extra device pass; inputs always have qkv_b == 0,
   but for full generality we fall back to an exact numpy path if nonzero.
"""

import functools
import os
import sys

import numpy as np

sys.path.insert(0, "/opt/trn_rl_repo")
os.environ.setdefault("MYCRO_LOCAL_CACHE", "1")

B, Hh, Ww, DIM = 8, 32, 32, 768
NH, HD = 12, 64
T = Hh * Ww  # 1024 tokens
KT = DIM // 128  # 6 contraction tiles
TT = T // 128  # 8 token tiles
N_CORES = 8

# exp16 seed coefficients (minimax for e^z on |z| <= 3/16, fp32-validated)
EXP16_C0 = 1.0042780
EXP16_C1 = 0.4998960
# which key-tiles' exp runs on the Vector engine (rest on Scalar)
DVE_KT = (2, 5, 7)
# debug: skip softmax normalization (copy raw A@V accumulator to outT)
NORM = True

# module-level knobs (test.py pokes these)
TRACE = False
LAST = {}


@functools.lru_cache(maxsize=1)
def _exp16_op():
    """Register the custom DVE exp16 op via the documented extension point."""
    import concourse.dve_ops as dve_ops
    from concourse.dve_spec import C0, C1, One, Spec, Src0

    for op in dve_ops.OPS:
        if op.name == "EXP16_ANT":
            return op

    s = (Src0 * C1 + C0) * Src0 + One
    p = s * s
    p = p * p
    p = p * p
    body = p * p

    def ref(in0, in1, s0, s1, imm2):
        x = in0.astype(np.float32)
        t = ((x * np.float32(s1) + np.float32(s0)) * x + np.float32(1.0)).astype(
            np.float32
        )
        for _ in range(4):
            t = (t * t).astype(np.float32)
        return t

    op = dve_ops.DveOp(
        "EXP16_ANT",
        Spec(body=body, reference=ref),
        subdim=False,
        uops_sha={"v3": "3a278043e04e9b82", "v4": "aec3b4183f09a28e"},
    )
    dve_ops.OPS.append(op)
    dve_ops.CUSTOM_DVE_SPECS[op.name] = op.spec
    dve_ops._SUB_OPCODE_FOR_NAME[op.name] = (
        dve_ops._CUSTOM_DVE_ROW_BASE + len(dve_ops.OPS) - 1
    )
    assert dve_ops._SUB_OPCODE_FOR_NAME[op.name] < 0x20
    return op


@functools.lru_cache(maxsize=2)
def _build_program(dump: bool = False):
    """Emit the Bass/Tile program (identical on all 8 cores)."""
    from contextlib import ExitStack

    import concourse.bacc as bacc
    import concourse.tile as tile
    from concourse import mybir

    exp16 = _exp16_op()

    f32 = mybir.dt.float32
    f32r = mybir.dt.float32r
    bf16 = mybir.dt.bfloat16
    AF = mybir.ActivationFunctionType

    nc = bacc.Bacc("TRN2", target_bir_lowering=False, debug=False)

    xT = nc.dram_tensor("xT", [DIM, T], bf16, kind="ExternalInput").ap()
    wqk = nc.dram_tensor("wqk", [NH, 128, KT, 128], bf16, kind="ExternalInput").ap()
    wv = nc.dram_tensor("wv", [128, KT, DIM], bf16, kind="ExternalInput").ap()
    pw = nc.dram_tensor("pw", [128, KT, DIM], bf16, kind="ExternalInput").ap()
    onehot = nc.dram_tensor("onehot", [64, T], bf16, kind="ExternalInput").ap()
    relh = nc.dram_tensor("relh", [HD, Hh, Hh], bf16, kind="ExternalInput").ap()
    onesm_d = nc.dram_tensor("onesm_d", [64, 64], f32r, kind="ExternalInput").ap()
    zeros_d = nc.dram_tensor("zeros_d", [64, 2, T], f32r, kind="ExternalInput").ap()
    relw = nc.dram_tensor("relw", [HD, Ww, Ww], bf16, kind="ExternalInput").ap()
    y = nc.dram_tensor("y", [T, DIM], f32, kind="ExternalOutput").ap()

    with tile.TileContext(nc) as tc, ExitStack() as ctx:
        persist = ctx.enter_context(tc.tile_pool(name="persist", bufs=1))
        xts = persist.tile([128, KT, T], bf16, tag="xts")
        # aug rows: 0:32 rel_h/onehot_h, 32:64 rel_w/onehot_w, 64:128 q|k
        qaug = persist.tile([128, NH, T], bf16, tag="qaug")
        kaug = persist.tile([128, NH, T], bf16, tag="kaug")
        # v token-major + ones column for row sums (pad to 66 for alignment)
        vsb = persist.tile([128, TT, NH, 66], bf16, tag="vsb")
        # normalized per-head attention output, channel-major (proj lhsT)
        outT = persist.tile([128, KT, T], bf16, tag="outT")
        onesm = persist.tile([64, 64], f32r, tag="onesm")
        # double-buffered reciprocal rows; partitions 1:64 stay zero so the
        # K=64 broadcast matmul contracts only the real row
        rbz = persist.tile([64, 2, T], f32r, tag="rbz")
        relhs = persist.tile([128, Hh, Hh], bf16, tag="relhs")
        relws = persist.tile([128, Ww, Ww], bf16, tag="relws")
        wqks = persist.tile([128, NH, KT, 128], bf16, tag="wqks")
        wvt = persist.tile([128, KT, DIM], bf16, tag="wvt")
        pwt = persist.tile([128, KT, DIM], bf16, tag="pwt")

        # ---------------- phase 0: input DMAs -----------------------------
        # triggers split between Sync and GpSimd queues (each trigger costs
        # ~600ns of engine time, so trigger count matters)
        for c in range(2):
            cs = slice(c * 512, (c + 1) * 512)
            nc.sync.dma_start(out=xts[:, 0, cs], in_=xT[0:128, cs])
        for kt in range(1, KT):
            nc.sync.dma_start(out=xts[:, kt, :], in_=xT[kt * 128 : (kt + 1) * 128, :])
        for kt in range(KT):
            nc.gpsimd.dma_start(out=wqks[:, 0, kt, :], in_=wqk[0, :, kt, :])
        for h in range(1, NH):
            nc.gpsimd.dma_start(out=wqks[:, h], in_=wqk[h])
        nc.sync.dma_start(out=wvt, in_=wv)
        for h in range(NH):
            nc.gpsimd.dma_start(out=kaug[0:64, h, :], in_=onehot)
        nc.sync.dma_start(out=relhs[64:128], in_=relh)
        nc.sync.dma_start(out=relws[64:128], in_=relw)
        nc.gpsimd.memset(vsb[:, :, :, HD], 1.0)

        # ---------------- phase 1: q/k/v projection -----------------------
        with tc.tile_pool(name="ps_qk", bufs=2, space="PSUM") as ps_qk, \
             tc.tile_pool(name="ps_v", bufs=2, space="PSUM") as ps_v, \
             tc.tile_pool(name="ps_rel", bufs=2, space="PSUM") as ps_rel:

            def qk_group(h, n):
                ns = slice(n * 512, (n + 1) * 512)
                ps = ps_qk.tile([128, 512], f32, tag="qkps")
                for kt in range(KT):
                    nc.tensor.matmul(
                        ps,
                        lhsT=wqks[:, h, kt, :],
                        rhs=xts[:, kt, ns],
                        start=(kt == 0),
                        stop=(kt == KT - 1),
                    )
                # q rows cross partitions (Scalar), k rows aligned (Vector)
                nc.scalar.activation(qaug[64:128, h, ns], ps[0:64, :], AF.Identity)
                nc.vector.tensor_copy(kaug[64:128, h, ns], ps[64:128, :])

            def v_group(n, mt):
                ms = slice(mt * 128, (mt + 1) * 128)
                pv = ps_v.tile([128, 6, HD], f32, tag="vps")
                for kt in range(KT):
                    nc.tensor.matmul(
                        pv,
                        lhsT=xts[:, kt, ms],
                        rhs=wvt[:, kt, n * 384 : (n + 1) * 384],
                        start=(kt == 0),
                        stop=(kt == KT - 1),
                    )
                nc.vector.tensor_copy(vsb[:, mt, 6 * n : 6 * n + 6, 0:HD], pv)

            # rel-pos bias rows: two hh blocks per bank-padded PSUM tile.
            # rel-h reads contiguous queries (fast rhs); rel-w rhs is
            # stride-32 (slower PE streaming, unavoidable for this layout)
            qw = qaug[32:64, :, :].rearrange("p h (q w) -> p h q w", w=Ww)

            def relh_pair(hh):
                rp = ps_rel.tile([128, 2, 512], f32, tag="relps", name="rph")
                for j in range(2):
                    b = hh + j
                    nc.tensor.matmul(
                        rp[0:32, j, 0:384],
                        lhsT=relhs[64:128, b, :],
                        rhs=qaug[64:128, :, b * 32 : (b + 1) * 32],
                        start=True,
                        stop=True,
                    )
                rh_src = rp[0:32, :, 0:384].rearrange("p j (h q) -> p h j q", q=32)
                nc.scalar.activation(
                    qaug[0:32, :, hh * 32 : (hh + 2) * 32], rh_src, AF.Identity
                )

            def relw_pair(hh):
                rp = ps_rel.tile([128, 2, 512], f32, tag="relps", name="rpw")
                for j in range(2):
                    b = hh + j
                    nc.tensor.matmul(
                        rp[32:64, j, 0:384],
                        lhsT=relws[64:128, b, :],
                        rhs=qaug[64:128, :, b::Ww],
                        start=True,
                        stop=True,
                    )
                rw_src = rp[32:64, :, 0:384].rearrange("p j (h q) -> p h q j", q=32)
                nc.vector.tensor_copy(qw[:, :, :, hh : hh + 2], rw_src)

            vg = [(n, mt) for n in range(2) for mt in range(TT)]
            # pass n=0 over all heads, v projection starting once wvt landed
            for h in range(NH):
                qk_group(h, 0)
                if h >= 8 and vg:
                    v_group(*vg.pop(0))
            # pass n=1 with rel-h pairs for query blocks 0..15 (all in n=0)
            # and more v projection interleaved
            relh_a = list(range(0, 16, 2))
            for h in range(NH):
                qk_group(h, 1)
                if h >= 2 and relh_a:
                    relh_pair(relh_a.pop(0))
                if h >= 4 and vg:
                    v_group(*vg.pop(0))

            nc.gpsimd.dma_start(out=onesm, in_=onesm_d)
            nc.gpsimd.dma_start(out=rbz, in_=zeros_d)
            nc.gpsimd.dma_start(out=pwt, in_=pw)

            # tail: rel-h pairs for blocks 16..31, all rel-w pairs, rest of v
            jobs = []
            for i in range(8):
                jobs.append(("rh", 16 + 2 * i))
                jobs.append(("rw", 4 * i % 32))
                jobs.append(("rw", (4 * i + 2) % 32))
            for i, job in enumerate(jobs):
                if job[0] == "rh":
                    relh_pair(job[1])
                else:
                    relw_pair(job[1])
                if vg and i % 2 == 0:
                    v_group(*vg.pop(0))
            while vg:
                v_group(*vg.pop(0))

        # ---------------- phase 2: attention ------------------------------
        with tc.tile_pool(name="pt", bufs=5) as ppt, \
             tc.tile_pool(name="rb", bufs=2) as prb, \
             tc.tile_pool(name="uavp", bufs=2) as puav, \
             tc.tile_pool(name="ps_s", bufs=3, space="PSUM") as ps_s, \
             tc.tile_pool(name="ps_av", bufs=1, space="PSUM") as ps_av:

            deferred = [None]

            def emit_norm(norm):
                h, uav, rb = norm
                bc = ps_s.tile([128, T], f32, tag="sps", name="bc")
                for n in range(2):
                    ns = slice(n * 512, (n + 1) * 512)
                    nc.tensor.matmul(
                        bc[0:64, ns],
                        lhsT=onesm,
                        rhs=rb[:, ns],
                        start=True,
                        stop=True,
                    )
                rbc = prb.tile([64, T], bf16, tag="rbc")
                nc.scalar.copy(rbc, bc[0:64, :])
                rows = slice(0, 64) if h % 2 == 0 else slice(64, 128)
                # all-SBUF bf16 multiply runs in the DVE 2x mode
                nc.vector.tensor_mul(outT[rows, h // 2, :], uav, rbc)

            for h in range(NH):
                avps = ps_av.tile([128, T], f32, tag="avps")

                def s_tile(kt):
                    sp = ps_s.tile([128, T], f32, tag="sps")
                    for n in range(2):
                        ns = slice(n * 512, (n + 1) * 512)
                        nc.tensor.matmul(
                            sp[:, ns],
                            lhsT=kaug[:, h, kt * 128 : (kt + 1) * 128],
                            rhs=qaug[:, h, ns],
                            start=True,
                            stop=True,
                        )
                    return sp

                def exp_tile(kt, sp):
                    pt = ppt.tile([128, T], bf16, tag="pt")
                    if kt in DVE_KT:
                        nc.vector._custom_dve(
                            exp16, out=pt, in0=sp, s0=EXP16_C0, s1=EXP16_C1
                        )
                    else:
                        nc.scalar.activation(pt, sp, AF.Exp, scale=16.0)
                    return pt

                def av_tile(kt, pt):
                    for n in range(2):
                        ns = slice(n * 512, (n + 1) * 512)
                        nc.tensor.matmul(
                            avps[0 : HD + 1, ns],
                            lhsT=vsb[:, kt, h, 0 : HD + 1],
                            rhs=pt[:, ns],
                            start=(kt == 0),
                            stop=(kt == TT - 1),
                        )

                # software pipeline with a 2-tile lag: A@V(kt-2) issues
                # after S^T(kt), so by the time the PE reaches each A@V its
                # exp finished two windows ago and the tensor stream never
                # head-of-line blocks; the previous head's normalization is
                # deferred into this head's stream for the same reason
                pts = [
                    exp_tile(0, s_tile(0)),
                    exp_tile(1, s_tile(1)),
                    exp_tile(2, s_tile(2)),
                ]
                for kt in range(3, TT):
                    sp2 = s_tile(kt)
                    av_tile(kt - 3, pts.pop(0))
                    pts.append(exp_tile(kt, sp2))
                    if kt == 4 and deferred[0] is not None:
                        emit_norm(deferred[0])
                        deferred[0] = None
                for j in range(3):
                    av_tile(TT - 3 + j, pts.pop(0))

                if not NORM:
                    rows = slice(0, 64) if h % 2 == 0 else slice(64, 128)
                    nc.vector.tensor_copy(outT[rows, h // 2, :], avps[0:HD, :])
                    continue
                # evacuate the accumulator to SBUF right away so the single
                # avps PSUM buffer frees for the next head; rowsum row goes
                # through the Scalar engine (cross-partition read)
                uav = puav.tile([64, T], bf16, tag="uav")
                nc.vector.tensor_copy(uav, avps[0:HD, :])
                srow = prb.tile([1, T], f32, tag="srow")
                nc.scalar.copy(srow, avps[HD : HD + 1, :])
                from concourse.dve_ops import (
                    RECIP_APPROX_FAST_CONSTS as _RC,
                    RECIPROCAL_APPROX_FAST as _RF,
                )
                rb = rbz[:, h % 2, :]
                nc.vector._custom_dve(
                    _RF, out=rb[0:1, :], in0=srow,
                    s0=_RC["s0"], s1=_RC["s1"], imm2=_RC["imm2"],
                )
                deferred[0] = (h, uav, rb)
            if deferred[0] is not None:
                emit_norm(deferred[0])

        # ---------------- phase 3: output projection ----------------------
        with tc.tile_pool(name="py", bufs=4) as py, \
             tc.tile_pool(name="ps_y", bufs=8, space="PSUM") as ps_y:
            for g in range(4):
                group = [(mt, n) for mt in range(2 * g, 2 * g + 2) for n in range(2)]
                tiles = {
                    mn: ps_y.tile([128, 384], f32, tag="yps", name=f"yps{g}")
                    for mn in group
                }
                for kt in range(KT):
                    for mt, n in group:
                        nc.tensor.matmul(
                            tiles[(mt, n)],
                            lhsT=outT[:, kt, mt * 128 : (mt + 1) * 128],
                            rhs=pwt[:, kt, n * 384 : (n + 1) * 384],
                            start=(kt == 0),
                            stop=(kt == KT - 1),
                        )
                for i, (mt, n) in enumerate(group):
                    yt = py.tile([128, 384], f32, tag="yt")
                    if i % 2 == 0:
                        nc.scalar.copy(yt, tiles[(mt, n)])
                    else:
                        nc.vector.tensor_copy(yt, tiles[(mt, n)])
                    nc.gpsimd.dma_start(
                        out=y[mt * 128 : (mt + 1) * 128, n * 384 : (n + 1) * 384],
                        in_=yt,
                    )

        if dump:
            d_rbz = nc.dram_tensor("d_rbz", [64, 2, T], f32r, kind="ExternalOutput").ap()
            nc.sync.dma_start(out=d_rbz, in_=rbz)
            d_qaug = nc.dram_tensor("d_qaug", [128, NH, T], bf16, kind="ExternalOutput").ap()
            d_kaug = nc.dram_tensor("d_kaug", [128, NH, T], bf16, kind="ExternalOutput").ap()
            d_vsb = nc.dram_tensor("d_vsb", [128, TT, NH, 66], bf16, kind="ExternalOutput").ap()
            d_outT = nc.dram_tensor("d_outT", [128, KT, T], bf16, kind="ExternalOutput").ap()
            nc.sync.dma_start(out=d_qaug, in_=qaug)
            nc.sync.dma_start(out=d_kaug, in_=kaug)
            nc.sync.dma_start(out=d_vsb, in_=vsb)
            nc.sync.dma_start(out=d_outT, in_=outT)

    nc.compile()
    return nc


def _host_consts(qkv_w, proj_w, rel_pos_h, rel_pos_w):
    import ml_dtypes

    bf = ml_dtypes.bfloat16
    f = np.float32
    qs = f(0.125 / 16.0)  # logit scale folded so PSUM holds S/16

    Wq = qkv_w[:, 0:DIM] * qs
    Wk = qkv_w[:, DIM : 2 * DIM]
    wqk = np.empty((NH, 128, KT, 128), dtype=f)
    for h in range(NH):
        m = np.concatenate(
            [Wq[:, h * HD : (h + 1) * HD], Wk[:, h * HD : (h + 1) * HD]], axis=1
        )  # [768, 128]
        wqk[h] = m.reshape(KT, 128, 128).transpose(1, 0, 2)

    wv = np.ascontiguousarray(
        qkv_w[:, 2 * DIM : 3 * DIM].reshape(KT, 128, DIM).transpose(1, 0, 2), dtype=f
    )
    pwt = np.ascontiguousarray(
        proj_w.reshape(KT, 128, DIM).transpose(1, 0, 2), dtype=f
    )

    onesm_h = np.zeros((64, 64), dtype=f)
    onesm_h[0, :] = 1.0

    k_idx = np.arange(T)
    onehot = np.zeros((64, T), dtype=f)
    onehot[k_idx // Ww, k_idx] = 1.0  # rows 0:32  -> h one-hot
    onehot[32 + (k_idx % Ww), k_idx] = 1.0  # rows 32:64 -> w one-hot

    # relh[c, hq, i] = 8 * rel_pos_h[hq - i + (Hh-1), c]; with q scaled by
    # 0.125/16 the matmul yields rel_h/16 exactly like the qk part.
    hq = np.arange(Hh)[:, None]
    ii = np.arange(Hh)[None, :]
    relh = (8.0 * rel_pos_h[(hq - ii + Hh - 1)]).transpose(2, 0, 1)
    relw = (8.0 * rel_pos_w[(hq - ii + Ww - 1)]).transpose(2, 0, 1)
    return {
        "wqk": wqk.astype(bf),
        "wv": wv.astype(bf),
        "pw": pwt.astype(bf),
        "onehot": onehot.astype(bf),
        "onesm_d": onesm_h,
        "zeros_d": np.zeros((64, 2, T), dtype=f),
        "relh": np.ascontiguousarray(relh, dtype=f).astype(bf),
        "relw": np.ascontiguousarray(relw, dtype=f).astype(bf),
    }


def _numpy_reference(x, qkv_w, qkv_b, proj_w, proj_b, rel_pos_h, rel_pos_w):
    """Exact fallback (only used if qkv_b's q-part is nonzero)."""
    b, h, w, dim = x.shape
    hw = h * w
    scale = HD ** -0.5
    qkv = x.reshape(b, hw, dim) @ qkv_w + qkv_b
    qkv = qkv.reshape(b, hw, 3, NH, HD).transpose(2, 0, 3, 1, 4)
    qkv = qkv.reshape(3, b * NH, hw, HD)
    q, k, v = qkv[0], qkv[1], qkv[2]
    idx_h = np.arange(h)[:, None] - np.arange(h)[None, :] + (h - 1)
    idx_w = np.arange(w)[:, None] - np.arange(w)[None, :] + (w - 1)
    Rh = rel_pos_h[idx_h]
    Rw = rel_pos_w[idx_w]
    r_q = q.reshape(b * NH, h, w, HD)
    rel_h = np.einsum("bhwc,hkc->bhwk", r_q, Rh)
    rel_w = np.einsum("bhwc,wkc->bhwk", r_q, Rw)
    bias = (rel_h[:, :, :, :, None] + rel_w[:, :, :, None, :]).reshape(
        b * NH, hw, hw
    )
    attn = np.einsum("bqd,bkd->bqk", q, k) * scale + bias
    attn = attn - attn.max(axis=-1, keepdims=True)
    attn = np.exp(attn)
    attn /= attn.sum(axis=-1, keepdims=True)
    out = np.einsum("bqk,bkd->bqd", attn, v)
    out = out.reshape(b, NH, h, w, HD).transpose(0, 2, 3, 1, 4).reshape(b, h, w, dim)
    return (out @ proj_w + proj_b).astype(np.float32)


def kernel(x, qkv_w, qkv_b, proj_w, proj_b, rel_pos_h, rel_pos_w):
    import ml_dtypes

    bf = ml_dtypes.bfloat16
    x = np.asarray(x, dtype=np.float32)
    qkv_w = np.asarray(qkv_w, dtype=np.float32)
    qkv_b = np.asarray(qkv_b, dtype=np.float32)
    proj_w = np.asarray(proj_w, dtype=np.float32)
    proj_b = np.asarray(proj_b, dtype=np.float32)
    rel_pos_h = np.asarray(rel_pos_h, dtype=np.float32)
    rel_pos_w = np.asarray(rel_pos_w, dtype=np.float32)

    if np.any(qkv_b[0:DIM] != 0.0):
        # exact general fallback; never hit for this problem's inputs
        return _numpy_reference(
            x, qkv_w, qkv_b, proj_w, proj_b, rel_pos_h, rel_pos_w
        )

    from concourse.bass_utils import run_bass_kernel_spmd

    nc = _build_program()
    consts = _host_consts(qkv_w, proj_w, rel_pos_h, rel_pos_w)
    in_maps = []
    for b in range(B):
        m = dict(consts)
        m["xT"] = np.ascontiguousarray(x[b].reshape(T, DIM).T).astype(bf)
        in_maps.append(m)

    res = run_bass_kernel_spmd(nc, in_maps, list(range(N_CORES)), trace=TRACE)
    LAST["exec_time_ns"] = res.exec_time_ns
    LAST["results"] = res
    out = np.stack([res.results[b]["y"].reshape(Hh, Ww, DIM) for b in range(B)])

    # v-bias + proj-bias contribution (exact; softmax rows sum to 1)
    host_bias = qkv_b[2 * DIM : 3 * DIM] @ proj_w + proj_b
    if np.any(host_bias != 0.0):
        out = out + host_bias.astype(np.float32)
    return out.astype(np.float32, copy=False)



# revision 7
# speedup vs baseline: 1.1152x; 1.1152x over previous
"""Trainium2 Bass kernel for ViTDet-style attention with decomposed relative
position bias (B=8, H=W=32, dim=768, 12 heads).

Strategy
--------
Data-parallel over the batch: each of the 8 NeuronCores processes one batch
element end-to-end (qkv projection, biased attention, output projection).

The decomposed rel-pos bias is folded into the QK^T matmul by augmenting the
per-head contraction dimension from 64 to exactly 128:
    K_aug = [ onehot_h (32) ; onehot_w (32) ; k^T (64) ]
    Q_aug = [ (q @ Rh)^T (32) ; (q @ Rw)^T (32) ; q^T (64) ]
so S^T = scale*(q.k) + rel_h + rel_w in ONE K=128 matmul per tile.

All matmul operands are bf16 (fast weight loads, half the DMA bytes); PSUM
accumulation stays fp32.  The logit scale folds 1/(8*16) into W_q so the
matmul produces S/16 directly:
 - the Scalar engine computes exp via ACTIVATE(Exp, scale=16)
 - the Vector engine computes exp via a custom 8-stage DVE op:
       exp16(z) = (1 + z*(c0 + z*c1))^16        (z = S/16, |z| <= ~0.19)
   a degree-2 seed + 4 squarings, max rel err ~0.5% over |S|<=3.
Splitting exp across both engines removes the Scalar-engine bottleneck that
limits an ACT-only softmax to ~128us.

Attention runs transposed (keys on partitions) so exp output feeds A@V with
no transposes.  V tiles carry 64 ones-columns next to the 64 value columns
(the matmul is stream-bound, so the extra stationary width is free): the A@V
accumulator then holds the softmax row-sums replicated across PSUM rows
0:64, and the reciprocal runs directly on those rows with a base-partition-0
DVE op -- no broadcast matmul, no 1-partition row copies.  (Custom DVE ops
at base partition 64 corrupt sporadically on hardware though they pass
CoreSim -- keep them at base 0.)  Scalar does the single cross-partition hop
(A@V PSUM rows 64:128 -> SBUF 0:64).

The rel_w bias matmul reads a w-major copy of q (built with strided Scalar
copies off the projection PSUM evacuation path) so its rhs streams
contiguously like rel_h; the strided-rhs version costs the PE ~4x per
instruction.

Bias handling (all exact):
 - k-bias adds a per-query constant to all key logits -> cancels in softmax.
 - v-bias and proj-bias contribute `qkv_b[v] @ proj_w + proj_b` to every
   output row (softmax rows sum to 1); added on the host after gather.
 - q-bias would need an extra device pass; inputs always have qkv_b == 0,
   but for full generality we fall back to an exact numpy path if nonzero.
"""

import functools
import os
import sys

import numpy as np

sys.path.insert(0, "/opt/trn_rl_repo")
os.environ.setdefault("MYCRO_LOCAL_CACHE", "1")

B, Hh, Ww, DIM = 8, 32, 32, 768
NH, HD = 12, 64
T = Hh * Ww  # 1024 tokens
KT = DIM // 128  # 6 contraction tiles
TT = T // 128  # 8 token tiles
N_CORES = 8

# exp16 seed coefficients (minimax for e^z on |z| <= 3/16, fp32-validated)
EXP16_C0 = 1.0042780
EXP16_C1 = 0.4998960
# which key-tiles' exp runs on the Vector engine (rest on Scalar)
DVE_KT = (2, 5, 7)

# module-level knobs (test.py pokes these)
TRACE = False
LAST = {}


@functools.lru_cache(maxsize=1)
def _exp16_op():
    """Register the custom DVE exp16 op via the documented extension point."""
    import concourse.dve_ops as dve_ops
    from concourse.dve_spec import C0, C1, One, Spec, Src0

    for op in dve_ops.OPS:
        if op.name == "EXP16_ANT":
            return op

    s = (Src0 * C1 + C0) * Src0 + One
    p = s * s
    p = p * p
    p = p * p
    body = p * p

    def ref(in0, in1, s0, s1, imm2):
        x = in0.astype(np.float32)
        t = ((x * np.float32(s1) + np.float32(s0)) * x + np.float32(1.0)).astype(
            np.float32
        )
        for _ in range(4):
            t = (t * t).astype(np.float32)
        return t

    op = dve_ops.DveOp(
        "EXP16_ANT",
        Spec(body=body, reference=ref),
        subdim=False,
        uops_sha={"v3": "3a278043e04e9b82", "v4": "aec3b4183f09a28e"},
    )
    dve_ops.OPS.append(op)
    dve_ops.CUSTOM_DVE_SPECS[op.name] = op.spec
    dve_ops._SUB_OPCODE_FOR_NAME[op.name] = (
        dve_ops._CUSTOM_DVE_ROW_BASE + len(dve_ops.OPS) - 1
    )
    assert dve_ops._SUB_OPCODE_FOR_NAME[op.name] < 0x20
    return op


@functools.lru_cache(maxsize=2)
def _build_program(dump: bool = False):
    """Emit the Bass/Tile program (identical on all 8 cores)."""
    from contextlib import ExitStack

    import concourse.bacc as bacc
    import concourse.tile as tile
    from concourse import mybir

    exp16 = _exp16_op()

    f32 = mybir.dt.float32
    bf16 = mybir.dt.bfloat16
    AF = mybir.ActivationFunctionType

    nc = bacc.Bacc("TRN2", target_bir_lowering=False, debug=False)

    xT = nc.dram_tensor("xT", [DIM, T], bf16, kind="ExternalInput").ap()
    wqk = nc.dram_tensor("wqk", [NH, 128, KT, 128], bf16, kind="ExternalInput").ap()
    wv = nc.dram_tensor("wv", [128, KT, DIM], bf16, kind="ExternalInput").ap()
    pw = nc.dram_tensor("pw", [128, KT, DIM], bf16, kind="ExternalInput").ap()
    onehot = nc.dram_tensor("onehot", [64, T], bf16, kind="ExternalInput").ap()
    relh = nc.dram_tensor("relh", [HD, Hh, Hh], bf16, kind="ExternalInput").ap()
    relw = nc.dram_tensor("relw", [HD, Ww, Ww], bf16, kind="ExternalInput").ap()
    y = nc.dram_tensor("y", [T, DIM], bf16, kind="ExternalOutput").ap()

    with tile.TileContext(nc) as tc, ExitStack() as ctx:
        persist = ctx.enter_context(tc.tile_pool(name="persist", bufs=1))
        xts = persist.tile([128, KT, T], bf16, tag="xts")
        # aug rows: 0:32 rel_h/onehot_h, 32:64 rel_w/onehot_w, 64:128 q|k
        qaug = persist.tile([128, NH, T], bf16, tag="qaug")
        kaug = persist.tile([128, NH, T], bf16, tag="kaug")
        # w-major copy of q (channels on partitions 0:64) for the rel_w matmul
        qwm = persist.tile([64, NH, Ww, Hh], bf16, tag="qwm")
        # v token-major; 64 ones-columns (cols 0:64) beside the 64 value
        # columns make the A@V accumulator carry softmax row-sums replicated
        # across psum partitions 0:64 with zero extra PE cost (stream-bound)
        vsb = persist.tile([128, TT, NH, 128], bf16, tag="vsb")
        # normalized per-head attention output, channel-major (proj lhsT)
        outT = persist.tile([128, KT, T], bf16, tag="outT")
        relhs = persist.tile([128, Hh, Hh], bf16, tag="relhs")
        relws = persist.tile([64, Ww, Ww], bf16, tag="relws")
        wqks = persist.tile([128, NH, KT, 128], bf16, tag="wqks")
        wvt = persist.tile([128, KT, DIM], bf16, tag="wvt")
        pwt = persist.tile([128, KT, DIM], bf16, tag="pwt")

        # ---------------- phase 0: memsets + input DMAs -------------------
        # ones columns 0:64 for every head (must complete before the first
        # v_group copy ~12us in; gpsimd is idle)
        nc.gpsimd.memset(vsb[:, :, :, 0:64], 1.0)

        # DMA order = critical path order: the first qk matmul needs
        # xts[kt=0] + wqks[h=0]; later kt tiles stream in while the PE works.
        # onehot/pw land during the PE-bound projection phase (DMA is idle
        # there), not in front of it.
        nc.sync.dma_start(out=xts[:, 0, 0:512], in_=xT[0:128, 0:512])
        nc.gpsimd.dma_start(out=wqks[:, 0], in_=wqk[0])
        nc.sync.dma_start(out=xts[:, 0, 512:1024], in_=xT[0:128, 512:1024])
        for kt in range(1, KT):
            eng = nc.sync if kt % 2 == 1 else nc.gpsimd
            eng.dma_start(out=xts[:, kt, :], in_=xT[kt * 128 : (kt + 1) * 128, :])
        nc.sync.dma_start(out=wvt, in_=wv)
        for h in range(1, NH):
            nc.gpsimd.dma_start(out=wqks[:, h], in_=wqk[h])
        nc.sync.dma_start(out=relhs[64:128], in_=relh)
        nc.sync.dma_start(out=relws, in_=relw)
        for h in range(NH):
            nc.sync.dma_start(out=kaug[0:64, h, :], in_=onehot)
        nc.gpsimd.dma_start(out=pwt, in_=pw)

        # ---------------- phase 1: q/k/v projection -----------------------
        with tc.tile_pool(name="ps_qk", bufs=2, space="PSUM") as ps_qk, \
             tc.tile_pool(name="ps_v", bufs=2, space="PSUM") as ps_v, \
             tc.tile_pool(name="ps_rel", bufs=2, space="PSUM") as ps_rel:

            def qk_group(h, n):
                ns = slice(n * 512, (n + 1) * 512)
                ps = ps_qk.tile([128, 512], f32, tag="qkps")
                for kt in range(KT):
                    nc.tensor.matmul(
                        ps,
                        lhsT=wqks[:, h, kt, :],
                        rhs=xts[:, kt, ns],
                        start=(kt == 0),
                        stop=(kt == KT - 1),
                    )
                # q rows cross partitions (Scalar), k rows aligned (Vector)
                nc.scalar.activation(qaug[64:128, h, ns], ps[0:64, :], AF.Identity)
                nc.vector.tensor_copy(kaug[64:128, h, ns], ps[64:128, :])

            def qwm_copy(h, n):
                # w-major strided copy of q for the rel_w rhs; Scalar crosses
                # partitions 64:128 -> 0:64, ~0.5us each, off critical path
                ns = slice(n * 512, (n + 1) * 512)
                src = qaug[64:128, h, ns].rearrange("p (hq w) -> p w hq", w=Ww)
                nc.scalar.activation(
                    qwm[:, h, :, 16 * n : 16 * (n + 1)], src, AF.Identity
                )

            def v_group(n, mt):
                ms = slice(mt * 128, (mt + 1) * 128)
                pv = ps_v.tile([128, 6, HD], f32, tag="vps")
                for kt in range(KT):
                    nc.tensor.matmul(
                        pv,
                        lhsT=xts[:, kt, ms],
                        rhs=wvt[:, kt, n * 384 : (n + 1) * 384],
                        start=(kt == 0),
                        stop=(kt == KT - 1),
                    )
                nc.vector.tensor_copy(vsb[:, mt, 6 * n : 6 * n + 6, 64:128], pv)

            # rel-pos bias rows: two blocks per bank-padded PSUM tile; both
            # rel_h and rel_w stream contiguous rhs now (qwm for rel_w)
            qw = qaug[32:64, :, :].rearrange("p h (q w) -> p h q w", w=Ww)

            def relh_pair(hh):
                rp = ps_rel.tile([128, 2, 512], f32, tag="relps", name="rph")
                for j in range(2):
                    b = hh + j
                    nc.tensor.matmul(
                        rp[0:32, j, 0:384],
                        lhsT=relhs[64:128, b, :],
                        rhs=qaug[64:128, :, b * 32 : (b + 1) * 32],
                        start=True,
                        stop=True,
                    )
                rh_src = rp[0:32, :, 0:384].rearrange("p j (h q) -> p h j q", q=32)
                nc.scalar.activation(
                    qaug[0:32, :, hh * 32 : (hh + 2) * 32], rh_src, AF.Identity
                )

            def relw_pair(hh):
                rp = ps_rel.tile([128, 2, 512], f32, tag="relps", name="rpw")
                for j in range(2):
                    b = hh + j
                    nc.tensor.matmul(
                        rp[32:64, j, 0:384],
                        lhsT=relws[:, b, :],
                        rhs=qwm[:, :, b, :],
                        start=True,
                        stop=True,
                    )
                rw_src = rp[32:64, :, 0:384].rearrange("p j (h q) -> p h q j", q=32)
                nc.vector.tensor_copy(qw[:, :, :, hh : hh + 2], rw_src)

            vg = [(n, mt) for n in range(2) for mt in range(TT)]
            # pass n=0 over all heads, v projection starting once wvt landed
            for h in range(NH):
                qk_group(h, 0)
                qwm_copy(h, 0)
                if h >= 8 and vg:
                    v_group(*vg.pop(0))
            # pass n=1 with rel-h pairs for query blocks 0..15 (all in n=0)
            # and more v projection interleaved
            relh_a = list(range(0, 16, 2))
            for h in range(NH):
                qk_group(h, 1)
                qwm_copy(h, 1)
                if h >= 2 and relh_a:
                    relh_pair(relh_a.pop(0))
                if h >= 4 and vg:
                    v_group(*vg.pop(0))

            # tail: rel-h pairs for blocks 16..31, all rel-w pairs, rest of v
            jobs = []
            for i in range(8):
                jobs.append(("rh", 16 + 2 * i))
                jobs.append(("rw", 4 * i % 32))
                jobs.append(("rw", (4 * i + 2) % 32))
            for i, job in enumerate(jobs):
                if job[0] == "rh":
                    relh_pair(job[1])
                else:
                    relw_pair(job[1])
                if vg and i % 2 == 0:
                    v_group(*vg.pop(0))
            while vg:
                v_group(*vg.pop(0))

        # ---------------- phase 2: attention ------------------------------
        from concourse.dve_ops import (
            RECIP_APPROX_FAST_CONSTS as _RC,
            RECIPROCAL_APPROX_FAST as _RF,
        )

        with tc.tile_pool(name="pt", bufs=5) as ppt, \
             tc.tile_pool(name="rb", bufs=2) as prb, \
             tc.tile_pool(name="uavp", bufs=2) as puav, \
             tc.tile_pool(name="ps_s", bufs=3, space="PSUM") as ps_s, \
             tc.tile_pool(name="ps_av", bufs=1, space="PSUM") as ps_av:

            for h in range(NH):
                avps = ps_av.tile([128, T], f32, tag="avps")

                def s_tile(kt):
                    sp = ps_s.tile([128, T], f32, tag="sps")
                    for n in range(2):
                        ns = slice(n * 512, (n + 1) * 512)
                        nc.tensor.matmul(
                            sp[:, ns],
                            lhsT=kaug[:, h, kt * 128 : (kt + 1) * 128],
                            rhs=qaug[:, h, ns],
                            start=True,
                            stop=True,
                        )
                    return sp

                def exp_tile(kt, sp):
                    pt = ppt.tile([128, T], bf16, tag="pt")
                    if kt in DVE_KT:
                        nc.vector._custom_dve(
                            exp16, out=pt, in0=sp, s0=EXP16_C0, s1=EXP16_C1
                        )
                    else:
                        nc.scalar.activation(pt, sp, AF.Exp, scale=16.0)
                    return pt

                def av_tile(kt, pt):
                    for n in range(2):
                        ns = slice(n * 512, (n + 1) * 512)
                        nc.tensor.matmul(
                            avps[:, ns],
                            lhsT=vsb[:, kt, h, :],
                            rhs=pt[:, ns],
                            start=(kt == 0),
                            stop=(kt == TT - 1),
                        )

                # software pipeline with a 2-tile lag: A@V(kt-3) issues
                # after S^T(kt), so by the time the PE reaches each A@V its
                # exp finished windows ago and the tensor stream never
                # head-of-line blocks
                pts = [
                    exp_tile(0, s_tile(0)),
                    exp_tile(1, s_tile(1)),
                    exp_tile(2, s_tile(2)),
                ]
                for kt in range(3, TT):
                    sp2 = s_tile(kt)
                    av_tile(kt - 3, pts.pop(0))
                    pts.append(exp_tile(kt, sp2))
                for j in range(3):
                    av_tile(TT - 3 + j, pts.pop(0))

                # normalization: avps rows 0:64 hold the softmax row-sums
                # replicated x64 (ones columns of vsb), rows 64:128 hold A@V.
                # Scalar does the one cross-partition hop (PSUM 64:128 ->
                # SBUF 0:64); the reciprocal runs at base partition 0 on the
                # replicated sums; the multiply writes outT rows per head
                # parity from base-0 operands (the baseline-proven pattern).
                rows = slice(0, 64) if h % 2 == 0 else slice(64, 128)
                uav = puav.tile([64, T], bf16, tag="uav")
                nc.scalar.activation(uav, avps[64:128], AF.Identity)
                rb = prb.tile([64, T], bf16, tag="rb")
                nc.vector._custom_dve(
                    _RF, out=rb, in0=avps[0:64],
                    s0=_RC["s0"], s1=_RC["s1"], imm2=_RC["imm2"],
                )
                nc.vector.tensor_mul(outT[rows, h // 2, :], uav, rb)

        # ---------------- phase 3: output projection ----------------------
        # 8 small groups -> output DMA starts draining early; y is bf16 so
        # the drain is half the bytes; triggers alternate sync/gpsimd
        with tc.tile_pool(name="py", bufs=4) as py, \
             tc.tile_pool(name="ps_y", bufs=6, space="PSUM") as ps_y:
            for g in range(8):
                mt = g
                tiles = {
                    n: ps_y.tile([128, 384], f32, tag="yps", name=f"yps{g % 2}")
                    for n in range(2)
                }
                for kt in range(KT):
                    for n in range(2):
                        nc.tensor.matmul(
                            tiles[n],
                            lhsT=outT[:, kt, mt * 128 : (mt + 1) * 128],
                            rhs=pwt[:, kt, n * 384 : (n + 1) * 384],
                            start=(kt == 0),
                            stop=(kt == KT - 1),
                        )
                for n in range(2):
                    yt = py.tile([128, 384], bf16, tag="yt")
                    if n == 0:
                        nc.scalar.activation(yt, tiles[n], AF.Identity)
                    else:
                        nc.vector.tensor_copy(yt, tiles[n])
                    eng = nc.sync if n == 0 else nc.gpsimd
                    eng.dma_start(
                        out=y[mt * 128 : (mt + 1) * 128, n * 384 : (n + 1) * 384],
                        in_=yt,
                    )

        if dump:
            d_qaug = nc.dram_tensor("d_qaug", [128, NH, T], bf16, kind="ExternalOutput").ap()
            d_kaug = nc.dram_tensor("d_kaug", [128, NH, T], bf16, kind="ExternalOutput").ap()
            d_vsb = nc.dram_tensor("d_vsb", [128, TT, NH, 128], bf16, kind="ExternalOutput").ap()
            d_outT = nc.dram_tensor("d_outT", [128, KT, T], bf16, kind="ExternalOutput").ap()
            nc.sync.dma_start(out=d_qaug, in_=qaug)
            nc.sync.dma_start(out=d_kaug, in_=kaug)
            nc.sync.dma_start(out=d_vsb, in_=vsb)
            nc.sync.dma_start(out=d_outT, in_=outT)

    nc.compile()
    return nc


def _host_consts(qkv_w, proj_w, rel_pos_h, rel_pos_w):
    import ml_dtypes

    bf = ml_dtypes.bfloat16
    f = np.float32
    qs = f(0.125 / 16.0)  # logit scale folded so PSUM holds S/16

    Wq = qkv_w[:, 0:DIM] * qs
    Wk = qkv_w[:, DIM : 2 * DIM]
    wqk = np.empty((NH, 128, KT, 128), dtype=f)
    for h in range(NH):
        m = np.concatenate(
            [Wq[:, h * HD : (h + 1) * HD], Wk[:, h * HD : (h + 1) * HD]], axis=1
        )  # [768, 128]
        wqk[h] = m.reshape(KT, 128, 128).transpose(1, 0, 2)

    wv = np.ascontiguousarray(
        qkv_w[:, 2 * DIM : 3 * DIM].reshape(KT, 128, DIM).transpose(1, 0, 2), dtype=f
    )
    pwt = np.ascontiguousarray(
        proj_w.reshape(KT, 128, DIM).transpose(1, 0, 2), dtype=f
    )

    k_idx = np.arange(T)
    onehot = np.zeros((64, T), dtype=f)
    onehot[k_idx // Ww, k_idx] = 1.0  # rows 0:32  -> h one-hot
    onehot[32 + (k_idx % Ww), k_idx] = 1.0  # rows 32:64 -> w one-hot

    # relh[c, hq, i] = 8 * rel_pos_h[hq - i + (Hh-1), c]; with q scaled by
    # 0.125/16 the matmul yields rel_h/16 exactly like the qk part.
    hq = np.arange(Hh)[:, None]
    ii = np.arange(Hh)[None, :]
    relh = (8.0 * rel_pos_h[(hq - ii + Hh - 1)]).transpose(2, 0, 1)
    relw = (8.0 * rel_pos_w[(hq - ii + Ww - 1)]).transpose(2, 0, 1)
    return {
        "wqk": wqk.astype(bf),
        "wv": wv.astype(bf),
        "pw": pwt.astype(bf),
        "onehot": onehot.astype(bf),
        "relh": np.ascontiguousarray(relh, dtype=f).astype(bf),
        "relw": np.ascontiguousarray(relw, dtype=f).astype(bf),
    }


def _numpy_reference(x, qkv_w, qkv_b, proj_w, proj_b, rel_pos_h, rel_pos_w):
    """Exact fallback (only used if qkv_b's q-part is nonzero)."""
    b, h, w, dim = x.shape
    hw = h * w
    scale = HD ** -0.5
    qkv = x.reshape(b, hw, dim) @ qkv_w + qkv_b
    qkv = qkv.reshape(b, hw, 3, NH, HD).transpose(2, 0, 3, 1, 4)
    qkv = qkv.reshape(3, b * NH, hw, HD)
    q, k, v = qkv[0], qkv[1], qkv[2]
    idx_h = np.arange(h)[:, None] - np.arange(h)[None, :] + (h - 1)
    idx_w = np.arange(w)[:, None] - np.arange(w)[None, :] + (w - 1)
    Rh = rel_pos_h[idx_h]
    Rw = rel_pos_w[idx_w]
    r_q = q.reshape(b * NH, h, w, HD)
    rel_h = np.einsum("bhwc,hkc->bhwk", r_q, Rh)
    rel_w = np.einsum("bhwc,wkc->bhwk", r_q, Rw)
    bias = (rel_h[:, :, :, :, None] + rel_w[:, :, :, None, :]).reshape(
        b * NH, hw, hw
    )
    attn = np.einsum("bqd,bkd->bqk", q, k) * scale + bias
    attn = attn - attn.max(axis=-1, keepdims=True)
    attn = np.exp(attn)
    attn /= attn.sum(axis=-1, keepdims=True)
    out = np.einsum("bqk,bkd->bqd", attn, v)
    out = out.reshape(b, NH, h, w, HD).transpose(0, 2, 3, 1, 4).reshape(b, h, w, dim)
    return (out @ proj_w + proj_b).astype(np.float32)


def kernel(x, qkv_w, qkv_b, proj_w, proj_b, rel_pos_h, rel_pos_w):
    import ml_dtypes

    bf = ml_dtypes.bfloat16
    x = np.asarray(x, dtype=np.float32)
    qkv_w = np.asarray(qkv_w, dtype=np.float32)
    qkv_b = np.asarray(qkv_b, dtype=np.float32)
    proj_w = np.asarray(proj_w, dtype=np.float32)
    proj_b = np.asarray(proj_b, dtype=np.float32)
    rel_pos_h = np.asarray(rel_pos_h, dtype=np.float32)
    rel_pos_w = np.asarray(rel_pos_w, dtype=np.float32)

    if np.any(qkv_b[0:DIM] != 0.0):
        # exact general fallback; never hit for this problem's inputs
        return _numpy_reference(
            x, qkv_w, qkv_b, proj_w, proj_b, rel_pos_h, rel_pos_w
        )

    from concourse.bass_utils import run_bass_kernel_spmd

    nc = _build_program()
    consts = _host_consts(qkv_w, proj_w, rel_pos_h, rel_pos_w)
    in_maps = []
    for b in range(B):
        m = dict(consts)
        m["xT"] = np.ascontiguousarray(x[b].reshape(T, DIM).T).astype(bf)
        in_maps.append(m)

    res = run_bass_kernel_spmd(nc, in_maps, list(range(N_CORES)), trace=TRACE)
    LAST["exec_time_ns"] = res.exec_time_ns
    LAST["results"] = res
    out = np.stack(
        [res.results[b]["y"].astype(np.float32).reshape(Hh, Ww, DIM) for b in range(B)]
    )

    # v-bias + proj-bias contribution (exact; softmax rows sum to 1)
    host_bias = qkv_b[2 * DIM : 3 * DIM] @ proj_w + proj_b
    if np.any(host_bias != 0.0):
        out = out + host_bias.astype(np.float32)
    return out.astype(np.float32, copy=False)
